# revision 7
# baseline (speedup 1.0000x reference)
"""MoE feed-forward (8 experts, top-2 routing) on 8 Trainium2 NeuronCores.

Strategy (expert parallelism):
  - Router runs on host with jax-CPU, replicating the reference's fp32 ops
    bit-for-bit (einsum + top_k + softmax) so expert selection matches.
  - Tokens are dispatched (gathered) per expert on host; each of the 8 cores
    runs one expert's SwiGLU FFN over its tokens:
        h = silu(x @ W1) * (x @ W2);  y = comb * (h @ W3)
    Stage 1 matmuls run as float32r (full PE rate, fp32 storage); h is stored
    bf16 in SBUF; stage 2 runs bf16 x bf16 with fp32 PSUM accumulation.
  - Host combines: out[token] += y_e rows (softmax weights already applied on
    device), plus the (comb @ b3) bias term.
"""

import sys
import types

for _p in ("/opt/trn_rl_repo", "/root/.axon_site/_ro/trn_rl_repo"):
    if _p not in sys.path:
        sys.path.append(_p)

import numpy as np
import ml_dtypes

import concourse.bass as bass
import concourse.mybir as mybir
import concourse.tile as tile
from concourse.bass_utils import run_bass_kernel_spmd

D_MODEL = 1024
D_FF = 4096
N_EXPERTS = 8
TOP_K = 2
P = 128
KO = D_MODEL // P  # 8 k-tiles over d_model
MF = D_FF // P  # 32 slices over d_ff

F32 = mybir.dt.float32
F32R = mybir.dt.float32r
BF16 = mybir.dt.bfloat16


# ---------------------------------------------------------------------------
# Workarounds for this container's toolchain
# ---------------------------------------------------------------------------
def _install_workarounds():
    # walrus here rejects >1 sync-wait on the TileContext-final Drain; split
    # the waits across a chain of single-wait drains.
    def _drain_and_barrier_split(self, tick_clock, wait_clock):
        drain_inst = self.nc.sync.drain()
        wait_clock.add_sem_waits(
            drain_inst.ins, tile.ScopedClock({None: tick_clock.global_clock})
        )
        si = drain_inst.ins.sync_info
        waits = list(si.on_wait) if si is not None else []
        if len(waits) > 1:
            si.on_wait = [waits[0]]
            for w in waits[1:]:
                d2 = self.nc.sync.drain()
                d2.ins.sync_info = mybir.SyncInfo(on_wait=[w], on_update=[])
        self.nc.all_engine_barrier()
        popped = self.nc._tile_sem_poison_stack.pop()
        assert popped is self._sem_poison
        self.nc.clear_and_free_semaphores(list(self.sems.allocated().values()))
        self.nc.all_engine_barrier()

    tile.TileContext._drain_and_barrier = _drain_and_barrier_split

    # antenv.axon_hooks is absent on this image; register the NTFF profile
    # hook from trn_agent_boot so trace=True works (no-op for trace=False).
    if "antenv.axon_hooks" not in sys.modules:
        try:
            from trn_agent_boot.trn_boot import _ntff_profile_via_ctypes

            hook = _ntff_profile_via_ctypes("/opt/axon/libaxon_pjrt.so")
        except Exception:
            hook = None
        mod = types.ModuleType("antenv.axon_hooks")
        mod.get_axon_ntff_profile_hook = lambda: hook
        mod.set_axon_ntff_profile_hook = lambda h: None
        sys.modules["antenv.axon_hooks"] = mod

    # artifact upload needs S3 creds we don't have; keep artifacts local.
    import concourse.bass_utils as bu

    bu.upload_artifacts = lambda tmpdir: "local://" + tmpdir

    # This walrus build accepts at most ONE sync-wait per non-DMA instruction
    # ("Too many sync wait commands"). Hoist extra waits onto single-wait
    # NoOps emitted just before the instruction on the same engine.
    import orjson

    def _split_multiwaits(bir: bytes) -> bytes:
        m = orjson.loads(bir)
        ctr = 0
        changed = False
        for f in m["functions"]:
            for blk in f["blocks"]:
                newinsts = []
                for inst in blk["instructions"]:
                    si = inst.get("sync_info")
                    if si and len(si.get("on_wait", [])) > 1:
                        waits = si["on_wait"]
                        for w in waits[:-1]:
                            ctr += 1
                            newinsts.append(
                                {
                                    "debug": inst.get("debug", 0),
                                    "engine": inst["engine"],
                                    "ins": [],
                                    "outs": [],
                                    "name": f"{inst['name']}_sw{ctr}",
                                    "opcode": "NoOp",
                                    "sync_info": {
                                        "on_wait": [w],
                                        "on_update": [],
                                    },
                                }
                            )
                        si["on_wait"] = [waits[-1]]
                        changed = True
                    newinsts.append(inst)
                blk["instructions"] = newinsts
        return orjson.dumps(m) if changed else bir

    _orig_tjb = bass.Bass.to_json_bytes

    def _to_json_bytes_split(self):
        return _split_multiwaits(_orig_tjb(self))

    bass.Bass.to_json_bytes = _to_json_bytes_split


_install_workarounds()


# ---------------------------------------------------------------------------
# Host-side router — replicates the reference router on jax-CPU
# ---------------------------------------------------------------------------
def _route(x, Wr, br):
    """Return comb [T, E] fp32 combine weights (0 for unselected experts) and
    top_idx [T, K] int — computed exactly as the reference does, on CPU."""
    import jax
    import jax.numpy as jnp

    cpu = jax.devices("cpu")[0]
    with jax.default_device(cpu):
        xj = jnp.asarray(np.asarray(x))
        logits = jnp.einsum("bsd,de->bse", xj, jnp.asarray(np.asarray(Wr)))
        logits = logits + jnp.asarray(np.asarray(br))
        top_vals, top_idx = jax.lax.top_k(logits, TOP_K)
        top_w = jax.nn.softmax(top_vals, axis=-1)
        comb = jnp.sum(
            jax.nn.one_hot(top_idx, N_EXPERTS, dtype=xj.dtype) * top_w[..., None],
            axis=-2,
        )
        comb_np = np.asarray(comb).reshape(-1, N_EXPERTS)
        idx_np = np.asarray(top_idx).reshape(-1, TOP_K)
    return comb_np, idx_np


def _token_blocks(tp):
    """Split tp (multiple of 128) into fp32r-friendly blocks (>=256 where
    possible, each <=512)."""
    n, r = divmod(tp, 512)
    if r == 0:
        blocks = [512] * n
    elif r == 128 and n >= 1:
        blocks = [512] * (n - 1) + [384, 256]
    else:
        blocks = [512] * n + [r]
    return blocks


# ---------------------------------------------------------------------------
# Device program (one expert per core, SPMD)
# ---------------------------------------------------------------------------
_prog_cache = {}


def _build_program(tp, stage1_f32r=True):
    """Bass program for one expert FFN over tp (padded) tokens."""
    nc = bass.Bass()
    s1dt = F32R if stage1_f32r else F32
    xT = nc.dram_tensor("xT", [D_MODEL, tp], s1dt, kind="ExternalInput")
    w1 = nc.dram_tensor("w1", [D_MODEL, D_FF], s1dt, kind="ExternalInput")
    w2 = nc.dram_tensor("w2", [D_MODEL, D_FF], s1dt, kind="ExternalInput")
    w3 = nc.dram_tensor("w3", [D_FF, D_MODEL], BF16, kind="ExternalInput")
    comb = nc.dram_tensor("comb", [P, tp // P], F32, kind="ExternalInput")
    y = nc.dram_tensor("y", [tp, D_MODEL], F32, kind="ExternalOutput")

    xT_r = xT.rearrange("(ko p) t -> p ko t", p=P)  # [128, 8, tp]
    w1_r = w1.rearrange("(ko p) f -> p ko f", p=P)  # [128, 8, 4096]
    w2_r = w2.rearrange("(ko p) f -> p ko f", p=P)
    w3_r = w3.rearrange("(k p) d -> p k d", p=P)  # [128, 32, 1024]

    NT = tp // P
    blocks = _token_blocks(tp)
    bmax = max(blocks)

    with tile.TileContext(nc) as tc:
        with (
            tc.tile_pool(name="persist", bufs=1) as persist,
            tc.tile_pool(name="w3p", bufs=1) as w3p,
            tc.tile_pool(name="wp", bufs=2) as wp,
            tc.tile_pool(name="sp", bufs=4) as sp,
            tc.tile_pool(name="yp", bufs=4) as yp,
            tc.tile_pool(name="psA", bufs=2, space="PSUM") as psA,
            tc.tile_pool(name="psB", bufs=2, space="PSUM") as psB,
            tc.tile_pool(name="psY", bufs=4, space="PSUM") as psY,
        ):
            # --- persistent SBUF tensors ---
            xT_sb = persist.tile([P, KO, tp], s1dt)
            h_sb = persist.tile([P, MF, tp], BF16)
            comb_sb = persist.tile([P, NT], F32)
            nc.sync.dma_start(comb_sb[:], comb[:])
            for ko in range(KO):
                nc.sync.dma_start(xT_sb[:, ko], xT_r[:, ko])

            # --- phase 1: h = silu(x@W1) * (x@W2), stored bf16 ---
            for m in range(MF):
                w1t = wp.tile([P, KO, P], s1dt, tag="w1t")
                w2t = wp.tile([P, KO, P], s1dt, tag="w2t")
                fsl = slice(m * P, (m + 1) * P)
                nc.sync.dma_start(w1t[:], w1_r[:, :, fsl])
                nc.sync.dma_start(w2t[:], w2_r[:, :, fsl])
                t0 = 0
                for nb in blocks:
                    tsl = slice(t0, t0 + nb)
                    ps1_full = psA.tile([P, bmax], F32, tag="ps1", name="ps1")
                    ps2_full = psB.tile([P, bmax], F32, tag="ps2", name="ps2")
                    ps1 = ps1_full[:, :nb]
                    ps2 = ps2_full[:, :nb]
                    for ko in range(KO):
                        nc.tensor.matmul(
                            ps1,
                            w1t[:, ko],
                            xT_sb[:, ko, tsl],
                            start=(ko == 0),
                            stop=(ko == KO - 1),
                        )
                    for ko in range(KO):
                        nc.tensor.matmul(
                            ps2,
                            w2t[:, ko],
                            xT_sb[:, ko, tsl],
                            start=(ko == 0),
                            stop=(ko == KO - 1),
                        )
                    sil_full = sp.tile([P, bmax], F32, tag="sil", name="sil")
                    sil = sil_full[:, :nb]
                    nc.scalar.activation(
                        sil, ps1, mybir.ActivationFunctionType.Silu
                    )
                    nc.vector.tensor_mul(h_sb[:, m, tsl], sil, ps2)
                    t0 += nb

            # --- phase 2: y = comb * (h @ W3), d_model in two halves ---
            for half in range(2):
                dsl = slice(half * 512, (half + 1) * 512)
                w3h = w3p.tile([P, MF, 512], BF16, tag="w3h")
                for kg in range(0, MF, 4):
                    nc.sync.dma_start(
                        w3h[:, kg : kg + 4], w3_r[:, kg : kg + 4, dsl]
                    )
                for t in range(NT):
                    psy = psY.tile([P, 512], F32, tag="psy")
                    tsl = slice(t * P, (t + 1) * P)
                    for k in range(MF):
                        nc.tensor.matmul(
                            psy,
                            h_sb[:, k, tsl],
                            w3h[:, k],
                            start=(k == 0),
                            stop=(k == MF - 1),
                        )
                    ysb = yp.tile([P, 512], F32, tag="ysb")
                    nc.vector.tensor_scalar_mul(ysb[:], psy, comb_sb[:, t : t + 1])
                    nc.sync.dma_start(y[tsl, dsl], ysb[:])
    return nc


def _get_program(tp, stage1_f32r=True):
    key = (tp, stage1_f32r)
    if key not in _prog_cache:
        _prog_cache[key] = _build_program(tp, stage1_f32r)
    return _prog_cache[key]


# ---------------------------------------------------------------------------
# Public entry point
# ---------------------------------------------------------------------------
def kernel(x, Wr, br, W1, b1, W2, b2, W3, b3):
    x = np.asarray(x)
    Wr = np.asarray(Wr)
    br = np.asarray(br)
    W1 = np.asarray(W1)
    b1 = np.asarray(b1)
    W2 = np.asarray(W2)
    b2 = np.asarray(b2)
    W3 = np.asarray(W3)
    b3 = np.asarray(b3)

    B, S, _ = x.shape
    T = B * S
    xf = np.ascontiguousarray(x.reshape(T, D_MODEL))

    if np.any(b1) or np.any(b2):
        raise NotImplementedError("nonzero b1/b2 not supported by this kernel")

    comb, top_idx = _route(x, Wr, br)

    # Dispatch: gather each expert's tokens (host all-to-all).
    sels = []
    for e in range(N_EXPERTS):
        sel = np.nonzero((top_idx == e).any(axis=1))[0]
        sels.append(sel)
    n_max = max(len(s) for s in sels)
    tp = max(512, -(-n_max // P) * P)  # pad to multiple of 128, >= 512

    in_maps = []
    for e in range(N_EXPERTS):
        sel = sels[e]
        n_e = len(sel)
        xT_e = np.zeros((D_MODEL, tp), dtype=np.float32)
        if n_e:
            xT_e[:, :n_e] = xf[sel].T
        comb_e = np.zeros(tp, dtype=np.float32)
        if n_e:
            comb_e[:n_e] = comb[sel, e]
        in_maps.append(
            {
                "xT": xT_e,
                "w1": np.ascontiguousarray(W1[e]),
                "w2": np.ascontiguousarray(W2[e]),
                "w3": np.ascontiguousarray(W3[e].astype(ml_dtypes.bfloat16)),
                "comb": np.ascontiguousarray(comb_e.reshape(tp // P, P).T),
            }
        )

    nc = _get_program(tp)
    res = run_bass_kernel_spmd(nc, in_maps, core_ids=list(range(N_EXPERTS)))

    # Combine: scatter-add weighted expert outputs (weights already applied).
    out = np.zeros((T, D_MODEL), dtype=np.float32)
    for e in range(N_EXPERTS):
        sel = sels[e]
        if len(sel):
            out[sel] += res.results[e]["y"][: len(sel)]
    if np.any(b3):
        out += comb @ b3
    return out.reshape(B, S, D_MODEL)
